# revision 53
# baseline (speedup 1.0000x reference)
"""Trainium2 Bass kernel for nn_Decoder (input proj -> relu RNN -> 2-layer head).

Strategy (8 NeuronCores, pure batch data-parallelism, 32 batch rows/core):
  - Fold the input projection into the recurrence drive on the host:
        f_t = W_rec @ ext_t + b_rec = W_eff @ x_t^T + b_eff
    with W_eff = W_rec @ W_in, b_eff = W_rec @ b_in + b_rec.  Then
        s_{t+1} = relu(W_rec @ s_t + f_t),   s_0 = 0.
  - x is cast to bf16 AND transposed on the host into GEMM-ready k-block
    layout (contraction dim s on partitions), halving HBM traffic vs fp32
    and eliminating all on-device TensorE transposes.  32 HWDGE DMAs of
    1 MiB each stream it on a dedicated SP ring (the model's per-core HBM
    roofline, ~93 us, is hit with <3% idle); all weights/biases arrive in
    ONE packed const DMA ahead of the stream.
  - ||W_rec||_2 ~ 0.31, so the recurrence forgets its state within ~16
    steps.  The 512-step chain is split into 8 CONCURRENT 64-step chains;
    chains 1-7 warm-start from zero 16 steps early.  One fused matmul per
    step serves all chains, and W_o1 rides along in the stationary for a
    free head tap:
        stationary [[W_rec^T, W_o1^T],[I, 0]] (128x96),
        rhs = [s_j ; f_j] (128, 8*32)
    so a step costs ONE matmul + ONE VectorE relu (state) + ONE ScalarE
    relu+bias (h tap, off the critical path).  F for a warm region is not
    recomputed: the owning chunk's PSUM is evicted twice (once to the
    owner chain's slot, once to the next chain's warm slot).
  - 8 bf16 GEMMs per chunk accumulate F = W_eff @ x^T into PSUM partitions
    64-127 (tile_position=(0,64)); a VectorE add evicts F (+b_eff) next to
    the state buffer so the fused step reads [s; f] with one access
    pattern.
  - The tensor engine clock ramps (0.65 -> 2.4 GHz) with sustained work
    and re-throttles after long idles: dummy warm-up matmuls + clock-
    keeper matmuls pinned behind the warm-phase chunk arrivals keep every
    real GEMM at full clock.
  - head2 (W_o2 @ h) runs per chunk from the banked h; outputs stage in
    per-stripe SBUF tiles and leave via one strided DMA per 8-chunk group
    on the ACT ring; b_o2 is added on the host and the channel-major
    output untransposed there.
  - Emission interleaves each stripe's F GEMMs and trailing head matmuls
    between the chain's step matmuls so the in-order PE queue fills the
    chain's relu-wait gaps; DMA stays ~16 chunks ahead so consecutive
    repeats pipeline (steady-state ~97 us/kernel in the cost model).
"""

import sys
import json
import numpy as np

for _p in ("/opt/trn_rl_repo",):
    if _p not in sys.path:
        sys.path.insert(0, _p)

import ml_dtypes
import concourse.bass as bass
import concourse.mybir as mybir
import concourse.tile as tile
from concourse.bass_utils import run_bass_kernel_spmd
from contextlib import ExitStack

BS, T, S, H = 256, 512, 1024, 64
NCORES = 8
B = BS // NCORES          # 32 batch rows per core
TC = 16                   # timesteps per chunk
NC_ = TC * B              # 512 columns (n = ti*B + b) per chunk
NCHUNK = T // TC          # 32 chunks per core
F32 = mybir.dt.float32
BF16 = mybir.dt.bfloat16

NCHAIN = 8                # concurrent chains
LCH = NCHUNK // NCHAIN    # 4 chunks (64 steps) per chain
W = TC                    # 16 warm-start steps (1 chunk) for chains 1..7
NSTEP = W + LCH * TC + 1  # 81 fused steps (last one only for its W_o1 tap)
SR = (NSTEP + 1) * B      # per-chain column stride in the state/F buffer
HSR = LCH * TC * B        # per-chain column stride in the head buffer

# chunk processing order: warm chunks (last chunk of chains 0..6) first,
# then stripe r = {4g + r}.  Chunks already loaded for warm-up are not
# reloaded (their PSUM is evicted to both destinations at load time).
SCHED = ([4 * g + 3 for g in range(7)]
         + [4 * g + 0 for g in range(8)]
         + [4 * g + 1 for g in range(8)]
         + [4 * g + 2 for g in range(8)]
         + [31])
NDMA = 16                 # 2 chunks (2 MiB) per DMA, alternating SP/Pool


def _split_multiwaits(nc, max_waits=1):
    """walrus in this container rejects >1 sem-wait on one instruction (the
    Tile end-of-kernel drain carries several).  Split extras into chained
    same-engine NoOps, then pin the serialized bytes on the nc object."""
    j = json.loads(nc.to_json_bytes())
    for f in j["functions"]:
        for bb in f["blocks"]:
            newinsts = []
            for inst in bb["instructions"]:
                si = inst.get("sync_info")
                waits = (si or {}).get("on_wait") or []
                if len(waits) > max_waits:
                    for k, w in enumerate(waits[max_waits:]):
                        newinsts.append({
                            "debug": inst.get("debug"),
                            "engine": inst["engine"],
                            "ins": [], "outs": [],
                            "name": f'{inst["name"]}-xw{k}',
                            "opcode": "NoOp",
                            "sync_info": {"on_update": [], "on_wait": [w]},
                        })
                    si["on_wait"] = waits[:max_waits]
                newinsts.append(inst)
            bb["instructions"] = newinsts
    b = json.dumps(j).encode()
    nc.to_json_bytes = lambda: b
    return nc


def build_decoder_nc(repeats=1):
    nc = bass.Bass("TRN2", target_bir_lowering=False, debug=False)

    # x host-packed: [16 dmas, 128 partitions, 2 * 8 k * 512 n] bf16
    x_d = nc.dram_tensor("x_shard", [NDMA, 128, 2 * 8 * NC_], BF16,
                         kind="ExternalInput")
    # all constants packed into one tensor / one DMA:
    #   cols 0:512    wpack[p, 64k+h] = W_eff[h, 128k+p]
    #   cols 512:608  [[W_rec^T, W_o1^T],[I_64, 0]] (step stationary)
    #   cols 608:610  W_o2^T (partitions 0:32)
    #   col  610      b_o1 (partitions 0:32), b_eff (partitions 64:128)
    cpack_d = nc.dram_tensor("cpack", [128, 611], BF16, kind="ExternalInput")
    out_d = nc.dram_tensor("out2", [2, T * B], BF16, kind="ExternalOutput")

    with tile.TileContext(nc) as tc:
        with ExitStack() as ctx:
            consts = ctx.enter_context(tc.tile_pool(name="consts", bufs=1))
            state_pool = ctx.enter_context(tc.tile_pool(name="state", bufs=1))
            xt_pool = ctx.enter_context(tc.tile_pool(name="xt", bufs=8))
            o_pool = ctx.enter_context(tc.tile_pool(name="obuf", bufs=2))
            f_ps_pool = ctx.enter_context(
                tc.tile_pool(name="f_ps", bufs=2, space="PSUM"))
            r_ps_pool = ctx.enter_context(
                tc.tile_pool(name="r_ps", bufs=3, space="PSUM"))
            o_ps_pool = ctx.enter_context(
                tc.tile_pool(name="o_ps", bufs=2, space="PSUM"))
            w_ps_pool = ctx.enter_context(
                tc.tile_pool(name="w_ps", bufs=1, space="PSUM"))

            # --- constants: ONE small HWDGE DMA ahead of the x stream ---
            cb = consts.tile([128, 611], BF16)
            nc.sync.dma_start(out=cb, in_=cpack_d.ap())
            wpack_sb = cb[:, 0:8 * H]
            wi_sb = cb[:, 8 * H:8 * H + H + 32]
            wo2t_sb = cb[0:32, 608:610]
            bias_sb = consts.tile([128, 1], F32)
            nc.vector.tensor_copy(bias_sb, cb[:, 610:611])
            bo1_sb = bias_sb[0:32, 0:1]
            beff_sb = bias_sb

            # state+drive buffer: partitions 0-63 hold s, 64-127 hold f.
            # chain g occupies cols [g*SR, (g+1)*SR):
            #   s_j at [0:64,  g*SR + j*B), j = 0..NSTEP
            #   f_j at [64:128, g*SR + j*B), j = 0..NSTEP-1
            sf = state_pool.tile([128, NCHAIN * SR], BF16)
            sf3 = sf.rearrange("p (g r) -> p g r", g=NCHAIN)
            for g in range(NCHAIN):
                nc.vector.memset(sf[0:64, g * SR:g * SR + B], 0.0)
            # chain 0 "warm" drive is exactly zero (state stays 0); the
            # final step's drive (only its W_o1 tap is used) is zero too
            nc.vector.memset(sf[64:128, 0:W * B], 0.0)
            nc.vector.memset(
                sf3[64:128, :, (NSTEP - 1) * B:NSTEP * B], 0.0)
            # head buffer (rolling 2-chunk window per chain):
            # h_{s_j} for chain g at col g*2*NC_ + ((j-W-1) % (2*TC))*B
            hb = state_pool.tile([32, NCHAIN * 2 * NC_], BF16)
            hb3 = hb.rearrange("p (g r) -> p g r", g=NCHAIN)
            # PE clock warm-up: the tensor engine ramps 0.65 -> 2.4 GHz with
            # ~3us of sustained work and re-throttles after idle gaps.  Burn
            # dummy matmuls on zeros while the first x chunks stream in so
            # real GEMMs run at full clock, and keep feeding filler between
            # the DMA-paced warm units so the clock never drops.
            wz = consts.tile([128, NC_], BF16)
            nc.vector.memset(wz, 0.0)
            wps = w_ps_pool.tile([64, NC_], F32)

            def emit_filler(n):
                for _ in range(n):
                    nc.tensor.matmul(wps, wz[:, 0:H], wz, start=True,
                                     stop=True)

            emit_filler(24)

            def make_ctx():
                return {"xt": {}, "dma": 0, "os": {}}

            def ensure_dma(st, d):
                d = min(d, NDMA - 1)
                while st["dma"] <= d:
                    i = st["dma"]
                    xtile = xt_pool.tile([128, 2 * 8 * NC_], BF16, tag="xt")
                    eng = nc.sync if i % 2 == 0 else nc.gpsimd
                    eng.dma_start(out=xtile, in_=x_d.ap()[i])
                    st["xt"][i] = xtile
                    st["dma"] = i + 1

            def emit_unit(st, u, filler=0):
                """F GEMM + eviction(s) for schedule position u."""
                ensure_dma(st, u // 2 + 2)
                q = SCHED[u]
                g, rc = q // LCH, q % LCH
                xtile = st["xt"][u // 2]
                off = (u % 2) * 8 * NC_
                fps = f_ps_pool.tile([128, NC_], F32, tag="fps")
                for k in range(8):
                    nc.tensor.matmul(
                        fps[64:128, :],
                        wpack_sb[:, k * H:(k + 1) * H],
                        xtile[:, off + k * NC_:off + (k + 1) * NC_],
                        start=(k == 0), stop=(k == 7),
                        tile_position=(0, 64))
                # clock-keeper matmuls pinned behind this unit's data so the
                # PE never idles long enough to re-throttle while the next
                # chunk streams in (junk results into the warm-up bank)
                for k in range(filler):
                    nc.tensor.matmul(wps, wz[:, 0:H],
                                     xtile[:, off + k * NC_:
                                           off + (k + 1) * NC_],
                                     start=True, stop=True)
                dst = sf[64:128, g * SR + (W + rc * TC) * B:
                         g * SR + (W + rc * TC) * B + NC_]
                nc.vector.tensor_scalar_add(dst, fps[64:128, :],
                                            beff_sb[64:128, 0:1])
                if rc == LCH - 1 and g < NCHAIN - 1:
                    dst2 = sf[64:128, (g + 1) * SR:(g + 1) * SR + NC_]
                    nc.vector.tensor_scalar_add(dst2, fps[64:128, :],
                                                beff_sb[64:128, 0:1])

            def emit_step(j):
                """One fused recurrence step for all chains.  The matmul
                also taps W_o1 @ s_j for free (extra stationary columns);
                ScalarE banks h = relu(W_o1 s_j + b_o1) off the critical
                path while VectorE applies the state relu."""
                rps = r_ps_pool.tile([96, NCHAIN * B], F32)
                nc.tensor.matmul(
                    rps, wi_sb,
                    sf3[:, :, j * B:(j + 1) * B],
                    start=True, stop=True)
                nc.vector.tensor_scalar_max(
                    sf3[0:64, :, (j + 1) * B:(j + 2) * B],
                    rps[0:64, :].rearrange("p (g r) -> p g r", g=NCHAIN),
                    0.0)
                if j > W:
                    m = (j - W - 1) % (2 * TC)
                    nc.scalar.activation(
                        hb3[:, :, m * B:(m + 1) * B],
                        rps[64:96, :].rearrange("p (g r) -> p g r",
                                                g=NCHAIN),
                        mybir.ActivationFunctionType.Relu,
                        bias=bo1_sb)

            # output DRAM viewed as [c, chain g, chunk rc, n] for group DMAs
            out4 = out_d.ap().rearrange("c (g rcq n) -> c g rcq n",
                                        g=NCHAIN, rcq=LCH)

            def emit_head(st, g, rc, half=None):
                """Output head for chain g's chunk rc (h already banked).
                Staged per-stripe; one strided DMA per 8-chunk group on the
                ACT HWDGE ring (its wait is satisfied at issue time)."""
                lo, n = 0, NC_
                if half is not None:
                    lo, n = half * (NC_ // 2), NC_ // 2
                op = o_ps_pool.tile([2, NC_], F32, tag="op")
                hbase = g * 2 * NC_ + (rc % 2) * NC_
                nc.tensor.matmul(
                    op[:, lo:lo + n], wo2t_sb,
                    hb[:, hbase + lo:hbase + lo + n],
                    start=True, stop=True)
                ent = st["os"].get(rc)
                if ent is None:
                    os_t = o_pool.tile([2, NCHAIN * NC_], BF16, tag="os")
                    ent = [os_t, 0]
                    st["os"][rc] = ent
                nc.scalar.copy(ent[0][:, g * NC_ + lo:g * NC_ + lo + n],
                               op[:, lo:lo + n])
                ent[1] += 1
                if ent[1] % NCHAIN == 0:
                    src = ent[0].rearrange("c (g n) -> c g n", g=NCHAIN)
                    if half is None:
                        nc.scalar.dma_start(out=out4[:, :, rc, :], in_=src)
                    else:
                        nc.scalar.dma_start(
                            out=out4[:, :, rc, lo:lo + n],
                            in_=src[:, :, lo:lo + n])

            for repi in range(repeats):
                st = make_ctx()
                ensure_dma(st, 2)
                for u in range(NCHAIN - 1):      # warm units
                    emit_unit(st, u, filler=5 if repi == 0 else 0)
                units = list(range(NCHAIN - 1, len(SCHED)))
                heads = []
                nu = nh = 0                       # consumed counts
                for j in range(NSTEP):
                    emit_step(j)
                    for rc in range(LCH - 1):
                        if j == 2 * TC + rc * TC:
                            heads += [(g, rc, None) for g in range(NCHAIN)]
                    if j == NSTEP - 9:            # last-chunk heads, 1st half
                        heads += [(g, LCH - 1, 0) for g in range(NCHAIN)]
                    # pace: all units done by step 48, heads trail stripes
                    want_u = min(len(units), (j + 1) * len(units) // 48)
                    while nu < want_u:
                        emit_unit(st, units[nu])
                        nu += 1
                    if nh < len(heads):
                        emit_head(st, *heads[nh])
                        nh += 1
                heads += [(g, LCH - 1, 1) for g in range(NCHAIN)]
                while nh < len(heads):
                    emit_head(st, *heads[nh])
                    nh += 1

    return _split_multiwaits(nc)


_NC_CACHE = None


def _get_nc():
    global _NC_CACHE
    if _NC_CACHE is None:
        _NC_CACHE = build_decoder_nc()
    return _NC_CACHE


def make_in_maps(inputs):
    x = np.asarray(inputs["x"], np.float32)
    W_in = np.asarray(inputs["W_in"], np.float32)
    b_in = np.asarray(inputs["b_in"], np.float32)
    W_rec = np.asarray(inputs["W_rec"], np.float32)
    b_rec = np.asarray(inputs["b_rec"], np.float32)
    W_o1 = np.asarray(inputs["W_o1"], np.float32)
    b_o1 = np.asarray(inputs["b_o1"], np.float32)
    W_o2 = np.asarray(inputs["W_o2"], np.float32)

    W_eff = (W_rec @ W_in).astype(np.float32)            # [64, 1024]
    b_eff = (W_rec @ b_in + b_rec).astype(np.float32)    # [64]

    bf = ml_dtypes.bfloat16
    wpack = np.zeros((128, 8 * H), bf)
    for k in range(8):
        wpack[:, k * H:(k + 1) * H] = W_eff[:, k * 128:(k + 1) * 128].T
    cpack = np.zeros((128, 611), bf)
    cpack[:, 0:8 * H] = wpack
    cpack[0:64, 8 * H:8 * H + H] = W_rec.T
    cpack[64:128, 8 * H:8 * H + H] = np.eye(64)
    cpack[0:64, 8 * H + H:8 * H + H + 32] = W_o1.T
    cpack[0:32, 608:610] = W_o2.T
    cpack[0:32, 610] = b_o1
    cpack[64:128, 610] = b_eff

    shared = {"cpack": cpack}
    xbf = x.astype(bf)
    sched = np.asarray(SCHED)
    in_maps = []
    for cid in range(NCORES):
        # [B, T, S] -> [q, p, (k ti b)]: chunk q, partition p = s % 128
        y = (xbf[cid * B:(cid + 1) * B]
             .reshape(B, NCHUNK, TC, 8, 128)
             .transpose(1, 4, 3, 2, 0)        # q, p, k, ti, b
             .reshape(NCHUNK, 128, 8 * NC_))
        y = (y[sched]
             .reshape(NDMA, 2, 128, 8 * NC_)
             .transpose(0, 2, 1, 3)
             .reshape(NDMA, 128, 2 * 8 * NC_))
        m = dict(shared)
        m["x_shard"] = np.ascontiguousarray(y)
        in_maps.append(m)
    return in_maps


def kernel(**inputs):
    b_o2 = np.asarray(inputs["b_o2"], np.float32)
    in_maps = make_in_maps(inputs)
    res = run_bass_kernel_spmd(_get_nc(), in_maps, core_ids=list(range(NCORES)))

    out = np.empty((BS, T, 2), np.float32)
    for cid in range(NCORES):
        o = np.asarray(res.results[cid]["out2"]).astype(np.float32)
        out[cid * B:(cid + 1) * B] = o.reshape(2, T, B).transpose(2, 1, 0)
    out += b_o2[None, None, :]
    return out


# revision 54
# speedup vs baseline: 1.0580x; 1.0580x over previous
"""Trainium2 Bass kernel for nn_Decoder (input proj -> relu RNN -> 2-layer head).

Strategy (8 NeuronCores, pure batch data-parallelism, 32 batch rows/core):
  - Fold the input projection into the recurrence drive on the host:
        f_t = W_rec @ ext_t + b_rec = W_eff @ x_t^T + b_eff
    with W_eff = W_rec @ W_in, b_eff = W_rec @ b_in + b_rec.  Then
        s_{t+1} = relu(W_rec @ s_t + f_t),   s_0 = 0.
  - x is cast to bf16 AND transposed on the host into GEMM-ready k-block
    layout (contraction dim s on partitions), halving HBM traffic vs fp32
    and eliminating all on-device TensorE transposes.  32 HWDGE DMAs of
    1 MiB each stream it on a dedicated SP ring (the model's per-core HBM
    roofline, ~93 us, is hit with <3% idle); all weights/biases arrive in
    ONE packed const DMA ahead of the stream.
  - ||W_rec||_2 ~ 0.31, so the recurrence forgets its state within ~16
    steps.  The 512-step chain is split into 8 CONCURRENT 64-step chains;
    chains 1-7 warm-start from zero 16 steps early.  One fused matmul per
    step serves all chains, and W_o1 rides along in the stationary for a
    free head tap:
        stationary [[W_rec^T, W_o1^T],[I, 0]] (128x96),
        rhs = [s_j ; f_j] (128, 8*32)
    so a step costs ONE matmul + ONE VectorE relu (state) + ONE ScalarE
    relu+bias (h tap, off the critical path).  F for a warm region is not
    recomputed: the owning chunk's PSUM is evicted twice (once to the
    owner chain's slot, once to the next chain's warm slot).
  - 8 bf16 GEMMs per chunk accumulate F = W_eff @ x^T into PSUM partitions
    64-127 (tile_position=(0,64)); a VectorE add evicts F (+b_eff) next to
    the state buffer so the fused step reads [s; f] with one access
    pattern.
  - The tensor engine clock ramps (0.65 -> 2.4 GHz) with sustained work
    and re-throttles after long idles: dummy warm-up matmuls + clock-
    keeper matmuls pinned behind the warm-phase chunk arrivals keep every
    real GEMM at full clock.
  - head2 (W_o2 @ h) runs per chunk from the banked h; outputs stage in
    per-stripe SBUF tiles and leave via one strided DMA per 8-chunk group
    on the ACT ring; b_o2 is added on the host and the channel-major
    output untransposed there.
  - Emission interleaves each stripe's F GEMMs and trailing head matmuls
    between the chain's step matmuls so the in-order PE queue fills the
    chain's relu-wait gaps; DMA stays ~16 chunks ahead so consecutive
    repeats pipeline (steady-state ~97 us/kernel in the cost model).
"""

import sys
import json
import numpy as np

for _p in ("/opt/trn_rl_repo",):
    if _p not in sys.path:
        sys.path.insert(0, _p)

import ml_dtypes
import concourse.bass as bass
import concourse.mybir as mybir
import concourse.tile as tile
from concourse.bass_utils import run_bass_kernel_spmd
from contextlib import ExitStack

BS, T, S, H = 256, 512, 1024, 64
NCORES = 8
B = BS // NCORES          # 32 batch rows per core
TC = 16                   # timesteps per chunk
NC_ = TC * B              # 512 columns (n = ti*B + b) per chunk
NCHUNK = T // TC          # 32 chunks per core
F32 = mybir.dt.float32
BF16 = mybir.dt.bfloat16

NCHAIN = 8                # concurrent chains
LCH = NCHUNK // NCHAIN    # 4 chunks (64 steps) per chain
W = TC                    # 16 warm-start steps (1 chunk) for chains 1..7
NSTEP = W + LCH * TC + 1  # 81 fused steps (last one only for its W_o1 tap)
SR = (NSTEP + 1) * B      # per-chain column stride in the state/F buffer
HSR = LCH * TC * B        # per-chain column stride in the head buffer

# chunk processing order: warm chunks (last chunk of chains 0..6) first,
# then stripe r = {4g + r}.  Chunks already loaded for warm-up are not
# reloaded (their PSUM is evicted to both destinations at load time).
SCHED = ([4 * g + 3 for g in range(7)]
         + [4 * g + 0 for g in range(8)]
         + [4 * g + 1 for g in range(8)]
         + [4 * g + 2 for g in range(8)]
         + [31])
NDMA = 64                 # half chunk (512 KiB) per DMA, alternating SP/Pool


def _split_multiwaits(nc, max_waits=1):
    """walrus in this container rejects >1 sem-wait on one instruction (the
    Tile end-of-kernel drain carries several).  Split extras into chained
    same-engine NoOps, then pin the serialized bytes on the nc object."""
    j = json.loads(nc.to_json_bytes())
    for f in j["functions"]:
        for bb in f["blocks"]:
            newinsts = []
            for inst in bb["instructions"]:
                si = inst.get("sync_info")
                waits = (si or {}).get("on_wait") or []
                if len(waits) > max_waits:
                    for k, w in enumerate(waits[max_waits:]):
                        newinsts.append({
                            "debug": inst.get("debug"),
                            "engine": inst["engine"],
                            "ins": [], "outs": [],
                            "name": f'{inst["name"]}-xw{k}',
                            "opcode": "NoOp",
                            "sync_info": {"on_update": [], "on_wait": [w]},
                        })
                    si["on_wait"] = waits[:max_waits]
                newinsts.append(inst)
            bb["instructions"] = newinsts
    b = json.dumps(j).encode()
    nc.to_json_bytes = lambda: b
    return nc


def build_decoder_nc(repeats=1):
    nc = bass.Bass("TRN2", target_bir_lowering=False, debug=False)

    # x host-packed: [64 dmas, 128 partitions, 4 k * 512 n] bf16
    x_d = nc.dram_tensor("x_shard", [NDMA, 128, 4 * NC_], BF16,
                         kind="ExternalInput")
    # all constants packed into one tensor / one DMA:
    #   cols 0:512    wpack[p, 64k+h] = W_eff[h, 128k+p]
    #   cols 512:608  [[W_rec^T, W_o1^T],[I_64, 0]] (step stationary)
    #   cols 608:610  W_o2^T (partitions 0:32)
    #   col  610      b_o1 (partitions 0:32), b_eff (partitions 64:128)
    cpack_d = nc.dram_tensor("cpack", [128, 611], BF16, kind="ExternalInput")
    out_d = nc.dram_tensor("out2", [2, T * B], BF16, kind="ExternalOutput")

    with tile.TileContext(nc) as tc:
        with ExitStack() as ctx:
            consts = ctx.enter_context(tc.tile_pool(name="consts", bufs=1))
            state_pool = ctx.enter_context(tc.tile_pool(name="state", bufs=1))
            xt_pool = ctx.enter_context(tc.tile_pool(name="xt", bufs=32))
            o_pool = ctx.enter_context(tc.tile_pool(name="obuf", bufs=2))
            f_ps_pool = ctx.enter_context(
                tc.tile_pool(name="f_ps", bufs=2, space="PSUM"))
            r_ps_pool = ctx.enter_context(
                tc.tile_pool(name="r_ps", bufs=3, space="PSUM"))
            o_ps_pool = ctx.enter_context(
                tc.tile_pool(name="o_ps", bufs=2, space="PSUM"))
            w_ps_pool = ctx.enter_context(
                tc.tile_pool(name="w_ps", bufs=1, space="PSUM"))

            # --- constants: ONE small HWDGE DMA ahead of the x stream ---
            cb = consts.tile([128, 611], BF16)
            nc.sync.dma_start(out=cb, in_=cpack_d.ap())
            wpack_sb = cb[:, 0:8 * H]
            wi_sb = cb[:, 8 * H:8 * H + H + 32]
            wo2t_sb = cb[0:32, 608:610]
            bias_sb = consts.tile([128, 1], F32)
            nc.vector.tensor_copy(bias_sb, cb[:, 610:611])
            bo1_sb = bias_sb[0:32, 0:1]
            beff_sb = bias_sb

            # state+drive buffer: partitions 0-63 hold s, 64-127 hold f.
            # chain g occupies cols [g*SR, (g+1)*SR):
            #   s_j at [0:64,  g*SR + j*B), j = 0..NSTEP
            #   f_j at [64:128, g*SR + j*B), j = 0..NSTEP-1
            sf = state_pool.tile([128, NCHAIN * SR], BF16)
            sf3 = sf.rearrange("p (g r) -> p g r", g=NCHAIN)
            for g in range(NCHAIN):
                nc.vector.memset(sf[0:64, g * SR:g * SR + B], 0.0)
            # chain 0 "warm" drive is exactly zero (state stays 0); the
            # final step's drive (only its W_o1 tap is used) is zero too
            nc.vector.memset(sf[64:128, 0:W * B], 0.0)
            nc.vector.memset(
                sf3[64:128, :, (NSTEP - 1) * B:NSTEP * B], 0.0)
            # head buffer (rolling 2-chunk window per chain):
            # h_{s_j} for chain g at col g*2*NC_ + ((j-W-1) % (2*TC))*B
            hb = state_pool.tile([32, NCHAIN * 2 * NC_], BF16)
            hb3 = hb.rearrange("p (g r) -> p g r", g=NCHAIN)
            # PE clock warm-up: the tensor engine ramps 0.65 -> 2.4 GHz with
            # ~3us of sustained work and re-throttles after idle gaps.  Burn
            # dummy matmuls on zeros while the first x chunks stream in so
            # real GEMMs run at full clock, and keep feeding filler between
            # the DMA-paced warm units so the clock never drops.
            wz = consts.tile([128, NC_], BF16)
            nc.vector.memset(wz, 0.0)
            wps = w_ps_pool.tile([64, NC_], F32)

            def emit_filler(n):
                for _ in range(n):
                    nc.tensor.matmul(wps, wz[:, 0:H], wz, start=True,
                                     stop=True)

            emit_filler(24)

            def make_ctx():
                return {"xt": {}, "dma": 0, "os": {}}

            def ensure_dma(st, d):
                d = min(d, NDMA - 1)
                while st["dma"] <= d:
                    i = st["dma"]
                    xtile = xt_pool.tile([128, 4 * NC_], BF16, tag="xt")
                    eng = nc.sync if i % 2 == 0 else nc.gpsimd
                    eng.dma_start(out=xtile, in_=x_d.ap()[i])
                    st["xt"][i] = xtile
                    st["dma"] = i + 1

            def emit_unit(st, u, filler=0):
                """F GEMM + eviction(s) for schedule position u."""
                ensure_dma(st, 2 * u + 5)
                q = SCHED[u]
                g, rc = q // LCH, q % LCH
                fps = f_ps_pool.tile([128, NC_], F32, tag="fps")
                for k in range(8):
                    xtile = st["xt"][2 * u + k // 4]
                    nc.tensor.matmul(
                        fps[64:128, :],
                        wpack_sb[:, k * H:(k + 1) * H],
                        xtile[:, (k % 4) * NC_:(k % 4 + 1) * NC_],
                        start=(k == 0), stop=(k == 7),
                        tile_position=(0, 64))
                # clock-keeper matmuls pinned behind this unit's data so the
                # PE never idles long enough to re-throttle while the next
                # chunk streams in (junk results into the warm-up bank)
                for k in range(filler):
                    xtile = st["xt"][2 * u + k // 4]
                    nc.tensor.matmul(wps, wz[:, 0:H],
                                     xtile[:, (k % 4) * NC_:
                                           (k % 4 + 1) * NC_],
                                     start=True, stop=True)
                dst = sf[64:128, g * SR + (W + rc * TC) * B:
                         g * SR + (W + rc * TC) * B + NC_]
                nc.vector.tensor_scalar_add(dst, fps[64:128, :],
                                            beff_sb[64:128, 0:1])
                if rc == LCH - 1 and g < NCHAIN - 1:
                    dst2 = sf[64:128, (g + 1) * SR:(g + 1) * SR + NC_]
                    nc.vector.tensor_scalar_add(dst2, fps[64:128, :],
                                                beff_sb[64:128, 0:1])

            def emit_step(j):
                """One fused recurrence step for all chains.  The matmul
                also taps W_o1 @ s_j for free (extra stationary columns);
                ScalarE banks h = relu(W_o1 s_j + b_o1) off the critical
                path while VectorE applies the state relu."""
                rps = r_ps_pool.tile([96, NCHAIN * B], F32)
                nc.tensor.matmul(
                    rps, wi_sb,
                    sf3[:, :, j * B:(j + 1) * B],
                    start=True, stop=True)
                nc.vector.tensor_scalar_max(
                    sf3[0:64, :, (j + 1) * B:(j + 2) * B],
                    rps[0:64, :].rearrange("p (g r) -> p g r", g=NCHAIN),
                    0.0)
                if j > W:
                    m = (j - W - 1) % (2 * TC)
                    nc.scalar.activation(
                        hb3[:, :, m * B:(m + 1) * B],
                        rps[64:96, :].rearrange("p (g r) -> p g r",
                                                g=NCHAIN),
                        mybir.ActivationFunctionType.Relu,
                        bias=bo1_sb)

            # output DRAM viewed as [c, chain g, chunk rc, n] for group DMAs
            out4 = out_d.ap().rearrange("c (g rcq n) -> c g rcq n",
                                        g=NCHAIN, rcq=LCH)

            def emit_head(st, g, rc, half=None):
                """Output head for chain g's chunk rc (h already banked).
                Staged per-stripe; one strided DMA per 8-chunk group on the
                ACT HWDGE ring (its wait is satisfied at issue time)."""
                lo, n = 0, NC_
                if half is not None:
                    lo, n = half * (NC_ // 2), NC_ // 2
                op = o_ps_pool.tile([2, NC_], F32, tag="op")
                hbase = g * 2 * NC_ + (rc % 2) * NC_
                nc.tensor.matmul(
                    op[:, lo:lo + n], wo2t_sb,
                    hb[:, hbase + lo:hbase + lo + n],
                    start=True, stop=True)
                ent = st["os"].get(rc)
                if ent is None:
                    os_t = o_pool.tile([2, NCHAIN * NC_], BF16, tag="os")
                    ent = [os_t, 0]
                    st["os"][rc] = ent
                nc.scalar.copy(ent[0][:, g * NC_ + lo:g * NC_ + lo + n],
                               op[:, lo:lo + n])
                ent[1] += 1
                if ent[1] % NCHAIN == 0:
                    src = ent[0].rearrange("c (g n) -> c g n", g=NCHAIN)
                    if half is None:
                        nc.scalar.dma_start(out=out4[:, :, rc, :], in_=src)
                    else:
                        nc.scalar.dma_start(
                            out=out4[:, :, rc, lo:lo + n],
                            in_=src[:, :, lo:lo + n])

            for repi in range(repeats):
                st = make_ctx()
                ensure_dma(st, 11)
                for u in range(NCHAIN - 1):      # warm units
                    emit_unit(st, u, filler=5 if repi == 0 else 0)
                units = list(range(NCHAIN - 1, len(SCHED)))
                heads = []
                nu = nh = 0                       # consumed counts
                for j in range(NSTEP):
                    emit_step(j)
                    for rc in range(LCH - 1):
                        if j == 2 * TC + rc * TC:
                            heads += [(g, rc, None) for g in range(NCHAIN)]
                    if j == NSTEP - 9:            # last-chunk heads, 1st half
                        heads += [(g, LCH - 1, 0) for g in range(NCHAIN)]
                    # pace: all units done by step 48, heads trail stripes
                    want_u = min(len(units), (j + 1) * len(units) // 48)
                    while nu < want_u:
                        emit_unit(st, units[nu])
                        nu += 1
                    if nh < len(heads):
                        emit_head(st, *heads[nh])
                        nh += 1
                heads += [(g, LCH - 1, 1) for g in range(NCHAIN)]
                while nh < len(heads):
                    emit_head(st, *heads[nh])
                    nh += 1

    return _split_multiwaits(nc)


_NC_CACHE = None


def _get_nc():
    global _NC_CACHE
    if _NC_CACHE is None:
        _NC_CACHE = build_decoder_nc()
    return _NC_CACHE


def make_in_maps(inputs):
    x = np.asarray(inputs["x"], np.float32)
    W_in = np.asarray(inputs["W_in"], np.float32)
    b_in = np.asarray(inputs["b_in"], np.float32)
    W_rec = np.asarray(inputs["W_rec"], np.float32)
    b_rec = np.asarray(inputs["b_rec"], np.float32)
    W_o1 = np.asarray(inputs["W_o1"], np.float32)
    b_o1 = np.asarray(inputs["b_o1"], np.float32)
    W_o2 = np.asarray(inputs["W_o2"], np.float32)

    W_eff = (W_rec @ W_in).astype(np.float32)            # [64, 1024]
    b_eff = (W_rec @ b_in + b_rec).astype(np.float32)    # [64]

    bf = ml_dtypes.bfloat16
    wpack = np.zeros((128, 8 * H), bf)
    for k in range(8):
        wpack[:, k * H:(k + 1) * H] = W_eff[:, k * 128:(k + 1) * 128].T
    cpack = np.zeros((128, 611), bf)
    cpack[:, 0:8 * H] = wpack
    cpack[0:64, 8 * H:8 * H + H] = W_rec.T
    cpack[64:128, 8 * H:8 * H + H] = np.eye(64)
    cpack[0:64, 8 * H + H:8 * H + H + 32] = W_o1.T
    cpack[0:32, 608:610] = W_o2.T
    cpack[0:32, 610] = b_o1
    cpack[64:128, 610] = b_eff

    shared = {"cpack": cpack}
    xbf = x.astype(bf)
    sched = np.asarray(SCHED)
    in_maps = []
    for cid in range(NCORES):
        # [B, T, S] -> [q, p, (k ti b)]: chunk q, partition p = s % 128
        y = (xbf[cid * B:(cid + 1) * B]
             .reshape(B, NCHUNK, TC, 8, 128)
             .transpose(1, 4, 3, 2, 0)        # q, p, k, ti, b
             .reshape(NCHUNK, 128, 8 * NC_))
        y = (y[sched]
             .reshape(32, 128, 2, 4 * NC_)
             .transpose(0, 2, 1, 3)
             .reshape(NDMA, 128, 4 * NC_))
        m = dict(shared)
        m["x_shard"] = np.ascontiguousarray(y)
        in_maps.append(m)
    return in_maps


def kernel(**inputs):
    b_o2 = np.asarray(inputs["b_o2"], np.float32)
    in_maps = make_in_maps(inputs)
    res = run_bass_kernel_spmd(_get_nc(), in_maps, core_ids=list(range(NCORES)))

    out = np.empty((BS, T, 2), np.float32)
    for cid in range(NCORES):
        o = np.asarray(res.results[cid]["out2"]).astype(np.float32)
        out[cid * B:(cid + 1) * B] = o.reshape(2, T, B).transpose(2, 1, 0)
    out += b_o2[None, None, :]
    return out
